# revision 4
# baseline (speedup 1.0000x reference)
"""Trainium2 Bass kernel for a 16-head dense attention block (B=1, S=2048, D=2048).

Sharding: 2 heads per core across 8 cores (tensor parallel on heads); the
reference's transpose(2,3)+reshape bug makes output rows [h*128:(h+1)*128)
depend only on head h, so per-core outputs are disjoint row blocks.

v2 design vs. baseline:
  - all DMA-heavy streams (x^T, wq/wk/wv, wo^T, rope consts) in bf16: halves
    HBM traffic; bf16 moving operands run 1 cycle/row at ANY free size
    (f32r needs >=256), enabling fine-grained causal skipping.
  - AV restructured with q on out-partitions: lhsT=exp-probs chunk (bf16
    stationary), rhs=V[k,dh] bf16 moving (128 rows) -- and the softmax
    denominator becomes a 1-cycle matmul against a ones column. This kills
    the rowsum ones-matmuls, the PE transposes, and the DRAM rowsum bounce
    of the baseline.
  - software pipeline: scores+exp for group g are emitted inside the
    projection blocks of group g (right after Q/K land), while the AV/rowsum
    matmuls for group g run during projection of group g+1, consuming exp
    outputs that are long since ready -- ScalarE latency never blocks PE.
  - Q/K projections process both heads dt-interleaved so the g=0 x-stream
    DMA chase never starves PE.
  - additive-mask chunks are deduplicated host-side (causal mask -> a single
    shared [128,128] tile).
"""

import math
import os

import numpy as np

S = 2048
D = 2048
H = 16
DH = 128
N_CORES = 8
HPC = H // N_CORES          # heads per core
NH = HPC * DH               # per-core head rows (256)
P = 128
QG = 512                    # q-group width
NQG = S // QG               # 4
NKT = S // P                # 16 k tiles
NDT = D // P                # 16 d tiles

_CACHE = {}


def _classify(maskT):
    """Per (kt, jq) 128x128 chunk of maskT[k,q]: skip if exp underflows to 0
    for the whole chunk, zero-mask if exactly zero, else masked (keyed by a
    dedup id so identical chunks share one SBUF tile).

    Returns (kinds, n_uniq): kinds[g][kt] is None (all chunks skip) or
    (off, chunk-ids) with chunk-ids a tuple over qc in [off,4) of either
    None (no mask) or a unique-chunk id."""
    uniq = {}
    kinds = []
    cls = np.empty((NKT, NKT), np.int8)
    for kt in range(NKT):
        for jq in range(NKT):
            blk = maskT[kt * P:(kt + 1) * P, jq * P:(jq + 1) * P]
            if np.all(blk <= -1e5):
                cls[kt, jq] = 0
            elif not blk.any():
                cls[kt, jq] = 1
            else:
                cls[kt, jq] = 2
    for g in range(NQG):
        row = []
        for kt in range(NKT):
            ck = [int(cls[kt, 4 * g + qc]) for qc in range(4)]
            if all(c == 0 for c in ck):
                row.append(None)
                continue
            off = 0
            while ck[off] == 0:
                off += 1
            ids = []
            for qc in range(off, 4):
                if ck[qc] == 1:
                    ids.append(None)
                else:  # masked (incl. interior all-skip: its mask zeroes it)
                    jq = 4 * g + qc
                    key = maskT[kt * P:(kt + 1) * P,
                                jq * P:(jq + 1) * P].tobytes()
                    ids.append(uniq.setdefault(hash(key),
                                               (len(uniq), kt, jq))[0])
            row.append((off, tuple(ids)))
        kinds.append(tuple(row))
    srcs = tuple((v[1], v[2]) for v in
                 sorted(uniq.values(), key=lambda v: v[0]))
    return tuple(kinds), srcs


def _build(kinds, mask_srcs):
    import concourse.tile as tile
    from concourse import bacc, mybir

    B = lambda k, d: int(os.environ.get(k, d))
    f32 = mybir.dt.float32
    bf16 = mybir.dt.bfloat16
    EXP = mybir.ActivationFunctionType.Exp

    nc = bacc.Bacc("TRN2", target_bir_lowering=False, debug=False,
                   num_devices=N_CORES)

    xT = nc.dram_tensor("xT", [D, S], bf16, kind="ExternalInput").ap()
    wqT = nc.dram_tensor("wqT", [D, NH], bf16, kind="ExternalInput").ap()
    wkT = nc.dram_tensor("wkT", [D, NH], bf16, kind="ExternalInput").ap()
    wvT = nc.dram_tensor("wvT", [D, NH], bf16, kind="ExternalInput").ap()
    maskT = nc.dram_tensor("maskT", [S, S], f32, kind="ExternalInput").ap()
    woT = nc.dram_tensor("woT", [S, D], bf16, kind="ExternalInput").ap()
    cq = nc.dram_tensor("cq", [DH, S], bf16, kind="ExternalInput").ap()
    sq = nc.dram_tensor("sq", [DH, S], bf16, kind="ExternalInput").ap()
    ck = nc.dram_tensor("ck", [DH, S], bf16, kind="ExternalInput").ap()
    sk = nc.dram_tensor("sk", [DH, S], bf16, kind="ExternalInput").ap()
    onesb = nc.dram_tensor("onesb", [P, 1], bf16, kind="ExternalInput").ap()
    perm = nc.dram_tensor("perm", [P, P], bf16, kind="ExternalInput").ap()
    out = nc.dram_tensor("out", [NH, D], f32, kind="ExternalOutput").ap()
    DBG = os.environ.get("KDBG")
    if DBG:
        dbg_qt = nc.dram_tensor("dbg_qt", [P, S], f32, kind="ExternalOutput").ap()
        dbg_kt = nc.dram_tensor("dbg_kt", [P, S], f32, kind="ExternalOutput").ap()
        dbg_vt = nc.dram_tensor("dbg_vt", [P, NKT * NH], f32, kind="ExternalOutput").ap()
        dbg_o = nc.dram_tensor("dbg_o", [P, S], f32, kind="ExternalOutput").ap()
        dbg_pt = nc.dram_tensor("dbg_pt", [P, S], f32, kind="ExternalOutput").ap()
        dbg_rs = nc.dram_tensor("dbg_rs", [NQG, P, HPC * 4], f32, kind="ExternalOutput").ap()
        dbg_av = nc.dram_tensor("dbg_av", [NQG, P, QG], f32, kind="ExternalOutput").ap()

    xT_v = xT.rearrange("(t p) s -> t p s", p=P)           # [16,128,S]
    wT_v = {"q": wqT.rearrange("(t p) n -> t p n", p=P),
            "k": wkT.rearrange("(t p) n -> t p n", p=P),
            "v": wvT.rearrange("(t p) n -> t p n", p=P)}
    maskT_v = maskT.rearrange("(t p) s -> t p s", p=P)
    woT_v = woT.rearrange("(t p) m -> t p m", p=P)

    # per-(g,qc) first/last active kt for AV psum start/stop flags
    first_kt = [[None] * 4 for _ in range(NQG)]
    last_kt = [[None] * 4 for _ in range(NQG)]
    for g in range(NQG):
        for qc in range(4):
            kts = [kt for kt in range(NKT)
                   if kinds[g][kt] is not None and kinds[g][kt][0] <= qc]
            assert kts, f"no active k for q-chunk ({g},{qc})"
            first_kt[g][qc] = kts[0]
            last_kt[g][qc] = kts[-1]

    with tile.TileContext(nc) as tc:
        with tc.tile_pool(name="consts", bufs=1) as consts, \
             tc.tile_pool(name="qkv", bufs=1) as qkv, \
             tc.tile_pool(name="xs_p", bufs=B("BX", 2)) as xs_p, \
             tc.tile_pool(name="rope_sb", bufs=B("BR", 2)) as rope_sb, \
             tc.tile_pool(name="pt_p", bufs=B("BP", 36)) as pt_p, \
             tc.tile_pool(name="rt_p", bufs=2) as rt_p, \
             tc.tile_pool(name="wo_sb", bufs=B("BW", 28)) as wo_sb, \
             tc.tile_pool(name="r_sb", bufs=2) as r_sb, \
             tc.tile_pool(name="ps_av", bufs=B("BAV", 1), space="PSUM") as ps_av, \
             tc.tile_pool(name="ps_rs", bufs=B("BRS", 1), space="PSUM") as ps_rs:
            ps_ctx = tc.tile_pool(name="ps", bufs=B("BPS", 5), space="PSUM")
            ps_pool = ps_ctx.__enter__()

            onesb_t = consts.tile([P, 1], bf16, tag="onesb")
            perm_t = consts.tile([P, P], bf16, tag="perm")
            rope_t = {nm: consts.tile([DH, S], bf16, tag=nm, name=nm)
                      for nm in ("cq", "sq", "ck", "sk")}
            w_t = {}
            for kind in ("q", "k", "v"):
                w_t[kind] = consts.tile([P, NDT, NH], bf16, tag=f"w{kind}",
                                        name=f"w{kind}")
            mask_t = []
            for i, (kt, jq) in enumerate(mask_srcs):
                mt = consts.tile([P, P], f32, tag=f"mt{i}", name=f"mt{i}")
                nc.gpsimd.dma_start(mt[:], maskT_v[kt][:, jq * P:(jq + 1) * P])
                mask_t.append(mt)

            qt = [qkv.tile([P, S], bf16, tag=f"qt{h}", name=f"qt{h}")
                  for h in range(HPC)]
            kt_ = [qkv.tile([P, S], bf16, tag=f"kt{h}", name=f"kt{h}")
                   for h in range(HPC)]
            vt = qkv.tile([P, NKT, NH], bf16, tag="v", name="vt")
            o_sb = [qkv.tile([P, S], bf16, tag=f"o{h}", name=f"o{h}")
                    for h in range(HPC)]

            # ---------------- emission helpers ----------------
            def emit_xs(g):
                """Load x^T chunks for group g on the sync queue. For g==0,
                interleave with wq chunks and split the first pieces finer so
                PE can start ASAP."""
                sl = slice(g * QG, (g + 1) * QG)
                xs_c = []
                for c in range(4):
                    cs = slice(c * 4, c * 4 + 4)
                    if g == 0:
                        wsrc = wT_v["q"][cs].rearrange("t p n -> p t n")
                        ksrc = wT_v["k"][cs].rearrange("t p n -> p t n")
                        if c == 0:
                            nc.sync.dma_start(w_t["q"][:, 0:1], wsrc[:, 0:1])
                            nc.sync.dma_start(w_t["k"][:, 0:1], ksrc[:, 0:1])
                        else:
                            if c == 1:
                                nc.sync.dma_start(perm_t[:], perm[:])
                            nc.sync.dma_start(w_t["q"][:, cs], wsrc)
                            nc.sync.dma_start(w_t["k"][:, cs], ksrc)
                    xc = xs_p.tile([P, 4, QG], bf16, tag=f"xs{c}",
                                   name=f"xs{c}")
                    src = xT_v[cs, :, sl].rearrange("t p s -> p t s")
                    if g == 0:
                        # halves: smoother arrival for the PE chase
                        if c == 0:
                            nc.sync.dma_start(xc[:, 0:1], src[:, 0:1])
                            wsrc = wT_v["q"][cs].rearrange("t p n -> p t n")
                            ksrc = wT_v["k"][cs].rearrange("t p n -> p t n")
                            nc.sync.dma_start(w_t["q"][:, 1:4], wsrc[:, 1:4])
                            nc.sync.dma_start(w_t["k"][:, 1:4], ksrc[:, 1:4])
                            nc.sync.dma_start(xc[:, 1:2], src[:, 1:2])
                            nc.sync.dma_start(xc[:, 2:4], src[:, 2:4])
                        else:
                            nc.sync.dma_start(xc[:, 0:2], src[:, 0:2])
                            nc.sync.dma_start(xc[:, 2:4], src[:, 2:4])
                    else:
                        nc.sync.dma_start(xc[:], src)
                    xs_c.append(xc)
                return xs_c

            def xs_at(xs_c, dt):
                return xs_c[dt // 4][:, dt % 4]

            def emit_qk_quad(g, xs_c, pump, pre=None):
                """Q and K projections for both heads, dt-interleaved (4 psum
                accumulators) so per-dt DMA demand stays under the DMA-engine
                bandwidth even in the cold-start window. Returns a closure
                applying RoPE to all four outputs, to be emitted inside the
                following V block."""
                sl = slice(g * QG, (g + 1) * QG)
                ps = {(kind, h): ps_pool.tile([P, QG], f32, tag="ps",
                                              name="psq")
                      for kind in ("q", "k") for h in range(HPC)}
                for dt in range(NDT):
                    if dt == 2 and pre is not None:
                        pre()
                    if dt % 2 == 0 and dt > 2:
                        pump()
                    for kind in ("q", "k"):
                        for h in range(HPC):
                            nc.tensor.matmul(
                                ps[(kind, h)][:],
                                w_t[kind][:, dt, h * P:(h + 1) * P],
                                xs_at(xs_c, dt), start=(dt == 0),
                                stop=(dt == NDT - 1))
                for kind, dst in (("q", qt), ("k", kt_)):
                    for h in range(HPC):
                        nc.vector.tensor_copy(dst[h][:, sl],
                                              ps[(kind, h)][:])

                def make_rope(kind, dst):
                    cn, sn = ("cq", "sq") if kind == "q" else ("ck", "sk")

                    def rope():
                        t1s, sws = [], []
                        for h in range(HPC):
                            # pair-swap via PE permutation matmul
                            ps_sw = ps_pool.tile([P, QG], f32, tag="ps",
                                                 name="ps_sw")
                            nc.tensor.matmul(ps_sw[:], perm_t[:],
                                             dst[h][:, sl],
                                             start=True, stop=True)
                            t1 = rope_sb.tile([P, QG], bf16, tag="t1",
                                              name="t1")
                            nc.vector.tensor_mul(t1[:], dst[h][:, sl],
                                                 rope_t[cn][:, sl])
                            sw = rope_sb.tile([P, QG], bf16, tag="sw",
                                              name="sw")
                            nc.vector.tensor_mul(sw[:], ps_sw[:],
                                                 rope_t[sn][:, sl])
                            t1s.append(t1)
                            sws.append(sw)
                        for h in range(HPC):
                            nc.vector.tensor_add(dst[h][:, sl], t1s[h],
                                                 sws[h])
                    return rope
                return make_rope("q", qt), make_rope("k", kt_)

            def emit_qk_pair(g, kind, xs_c, pump, pre=None):
                """Q or K projection for both heads, dt-interleaved; RoPE
                returned as a deferred closure."""
                sl = slice(g * QG, (g + 1) * QG)
                dst = qt if kind == "q" else kt_
                cn, sn = ("cq", "sq") if kind == "q" else ("ck", "sk")
                ps = [ps_pool.tile([P, QG], f32, tag="ps", name="psq")
                      for _ in range(HPC)]
                for dt in range(NDT):
                    if dt == 2 and pre is not None:
                        pre()
                    if dt % 2 == 0 and dt > 2:
                        pump()
                    for h in range(HPC):
                        nc.tensor.matmul(ps[h][:],
                                         w_t[kind][:, dt, h * P:(h + 1) * P],
                                         xs_at(xs_c, dt), start=(dt == 0),
                                         stop=(dt == NDT - 1))
                for h in range(HPC):
                    nc.vector.tensor_copy(dst[h][:, sl], ps[h][:])

                def rope():
                    t1s, sws = [], []
                    for h in range(HPC):
                        ps_sw = ps_pool.tile([P, QG], f32, tag="ps",
                                             name="ps_sw")
                        nc.tensor.matmul(ps_sw[:], perm_t[:], dst[h][:, sl],
                                         start=True, stop=True)
                        t1 = rope_sb.tile([P, QG], bf16, tag="t1", name="t1")
                        nc.vector.tensor_mul(t1[:], dst[h][:, sl],
                                             rope_t[cn][:, sl])
                        sw = rope_sb.tile([P, QG], bf16, tag="sw", name="sw")
                        nc.vector.tensor_mul(sw[:], ps_sw[:],
                                             rope_t[sn][:, sl])
                        t1s.append(t1)
                        sws.append(sw)
                    for h in range(HPC):
                        nc.vector.tensor_add(dst[h][:, sl], t1s[h], sws[h])
                return rope

            def emit_v_pair(g, pair, xs_c, pump, pres=()):
                pres = list(pres)
                for st_l in pair:
                    st = g * 4 + st_l
                    lsl = slice(st_l * P, st_l * P + P)
                    ps = ps_pool.tile([P, NH], f32, tag="ps", name="psv")
                    for dt in range(NDT):
                        if dt == 2 and pres:
                            pres.pop(0)()
                        if dt % 4 == 0 and dt > 0:
                            pump()
                        nc.tensor.matmul(ps[:], xs_at(xs_c, dt)[:, lsl],
                                         w_t["v"][:, dt], start=(dt == 0),
                                         stop=(dt == NDT - 1))
                    nc.vector.tensor_copy(vt[:, st], ps[:])
                    pump()

            class Attn:
                """Attention for group g in decoupled micro-steps:
                - sc steps (one per (kt,h)): scores matmul + mask add + exp
                  into a bf16 probs tile; emitted during proj(g).
                - av steps (one per kt): AV + rowsum matmuls for both heads;
                  emitted during proj(g+1), when exps are long done.
                - epi: reciprocal + normalize into o_sb."""

                def __init__(self, g):
                    self.g = g
                    self.row = kinds[g]
                    self.kts = [kt for kt in range(NKT)
                                if self.row[kt] is not None]
                    self.sc_seq = [(kt, h) for kt in self.kts
                                   for h in range(HPC)]
                    self.av_seq = list(self.kts)
                    self.sc_i = 0
                    self.av_i = 0
                    self.stage = 0  # 0: none, 1: kt<4g, 2: all
                    self.pt = {}
                    self.av = None
                    # PSUM zero-region semantics: start=True marks the WHOLE
                    # 2KB region pending-zero, so only the first matmul into
                    # each psum tile may carry it; later first-writes to
                    # still-pending bytes overwrite (= implicit start).
                    self.av_started = [False] * HPC
                    self.rs_started = False

                def sc_avail(self):
                    if self.stage == 0 or self.sc_i >= len(self.sc_seq):
                        return False
                    kt, _ = self.sc_seq[self.sc_i]
                    return self.stage >= 2 or kt < 4 * self.g

                def sc_step(self):
                    kt, h = self.sc_seq[self.sc_i]
                    self.sc_i += 1
                    g = self.g
                    off, ids = self.row[kt]
                    osl = slice(off * P, QG)
                    qsl = slice(g * QG + off * P, (g + 1) * QG)
                    ksl = slice(kt * P, (kt + 1) * P)
                    sc = ps_pool.tile([P, QG], f32, tag="ps", name="sc")
                    nc.tensor.matmul(sc[:, osl], kt_[h][:, ksl],
                                     qt[h][:, qsl], start=True, stop=True)
                    for idx, mid in enumerate(ids):
                        if mid is not None:
                            qc = off + idx
                            csl = slice(qc * P, (qc + 1) * P)
                            nc.vector.tensor_add(sc[:, csl], sc[:, csl],
                                                 mask_t[mid][:])
                    pt = pt_p.tile([P, QG], bf16, tag="pt", name="pt")
                    nc.scalar.activation(pt[:, osl], sc[:, osl], EXP)
                    self.pt[(kt, h)] = pt

                def av_avail(self):
                    if self.av_i >= len(self.av_seq):
                        return False
                    need = min((self.av_i + 2) * HPC, len(self.sc_seq))
                    return self.sc_i >= need

                def av_step(self):
                    kt = self.av_seq[self.av_i]
                    self.av_i += 1
                    g = self.g
                    if self.av is None:
                        self.av = [ps_av.tile([P, QG], f32, tag=f"av{h}",
                                              name=f"av{h}")
                                   for h in range(HPC)]
                        self.rs = ps_rs.tile([P, HPC * 4], f32, tag="rs",
                                             name="rs")
                    off, _ = self.row[kt]
                    for h in range(HPC):
                        pt = self.pt.pop((kt, h))
                        for qc in range(off, 4):
                            qcsl = slice(qc * P, (qc + 1) * P)
                            sp = (kt == last_kt[g][qc])
                            nc.tensor.matmul(self.av[h][:, qcsl],
                                             pt[:, qcsl],
                                             vt[:, kt, h * P:(h + 1) * P],
                                             start=not self.av_started[h],
                                             stop=sp, skip_group_check=True)
                            self.av_started[h] = True
                            col = h * 4 + qc
                            nc.tensor.matmul(self.rs[:, col:col + 1],
                                             pt[:, qcsl], onesb_t[:],
                                             start=not self.rs_started,
                                             stop=sp, skip_group_check=True)
                            self.rs_started = True

                def done(self):
                    return self.av_i >= len(self.av_seq)

                def epi(self):
                    g = self.g
                    rt = rt_p.tile([P, HPC * 4], f32, tag="rt", name="rt")
                    if DBG:
                        drs = rt_p.tile([P, HPC * 4], f32, tag="drs", name="drs")
                        nc.vector.tensor_copy(drs[:], self.rs[:])
                        nc.scalar.dma_start(dbg_rs[g], drs[:])
                        dav = pt_p.tile([P, QG], f32, tag="dav", name="dav")
                        nc.vector.tensor_copy(dav[:], self.av[0][:])
                        nc.scalar.dma_start(dbg_av[g], dav[:])
                    nc.vector.reciprocal(rt[:], self.rs[:])
                    for h in range(HPC):
                        for qc in range(4):
                            jq = 4 * g + qc
                            jsl = slice(jq * P, (jq + 1) * P)
                            nc.scalar.mul(o_sb[h][:, jsl],
                                          self.av[h][:, qc * P:(qc + 1) * P],
                                          rt[:, h * 4 + qc:h * 4 + qc + 1])

            # output projection helpers
            wo_tiles = {}

            def load_wo(mg):
                msl = slice(mg * QG, (mg + 1) * QG)
                for pr in range(NKT // 2):
                    t = wo_sb.tile([P, 2, QG], bf16, tag="wo", name="wt")
                    nc.sync.dma_start(
                        t[:],
                        woT_v[2 * pr:2 * pr + 2, :, msl].rearrange(
                            "t p m -> p t m"))
                    wo_tiles[(mg, pr)] = t

            def ph4_wave(mg, h, pump=None, cols=None):
                """cols: optional column sub-range of the output group —
                used to break the final wave into sub-waves whose output
                stores overlap the next sub-wave's matmuls."""
                c0, c1 = (0, QG) if cols is None else cols
                csl = slice(c0, c1)
                psr = ps_r.tile([P, QG], f32, tag="psr", name="psr")
                rs = r_sb.tile([P, QG], f32, tag="rso", name="rso")
                for jt in range(NKT):
                    if jt == 12 and pump is not None:
                        pump()
                    jsl = slice(jt * P, (jt + 1) * P)
                    wt = wo_tiles[(mg, jt // 2)][:, jt % 2][:, csl]
                    nc.tensor.matmul(psr[:, csl], o_sb[h][:, jsl], wt,
                                     start=(jt == 0), stop=(jt == NKT - 1))
                osl = slice(mg * QG + c0, mg * QG + c1)
                nc.vector.tensor_copy(rs[:, csl], psr[:, csl])
                nc.sync.dma_start(out[h * P:(h + 1) * P, osl], rs[:, csl])

            # ---------------- main schedule ----------------
            # everything on the sync queue, ordered by first-need time
            def load_consts(step):
                if step == 0:      # V weights + rope tables, by need
                    nc.sync.dma_start(
                        w_t["v"][:, 0:8],
                        wT_v["v"][0:8].rearrange("t p n -> p t n"))
                    nc.sync.dma_start(rope_t["cq"][:], cq[:])
                    nc.sync.dma_start(
                        w_t["v"][:, 8:16],
                        wT_v["v"][8:16].rearrange("t p n -> p t n"))
                    nc.sync.dma_start(rope_t["sq"][:], sq[:])
                else:
                    nc.sync.dma_start(rope_t["ck"][:], ck[:])
                    nc.sync.dma_start(rope_t["sk"][:], sk[:])
                    nc.sync.dma_start(onesb_t[:], onesb[:])

            att_prev = None
            att_cur = None
            pending_rope = [None]

            def make_pump(sc_att, av_att, n_av, allow_prev_sc=True):
                """Per call: a scores-step, AV-steps, another scores-step.
                The AV matmuls between the two sc steps give the first sc's
                exp time to drain before the second reuses its psum slot.
                allow_prev_sc=False keeps the previous group's spilled scores
                out of this block: their rope dep would park them in PE's
                4-deep wait queue and clog dispatch."""
                def sc_one():
                    if (allow_prev_sc and av_att is not None
                            and av_att.sc_avail()):
                        av_att.sc_step()
                    elif sc_att is not None and sc_att.sc_avail():
                        sc_att.sc_step()
                    else:
                        return False
                    return True

                def pump():
                    sc_one()
                    did_av = False
                    if av_att is not None:
                        for _ in range(n_av):
                            if av_att.av_avail():
                                av_att.av_step()
                                did_av = True
                    if did_av:
                        sc_one()
                return pump

            for g in range(NQG):
                xs_c = emit_xs(g)
                if g == 0:
                    load_consts(0)
                elif g == 1:
                    load_consts(1)
                elif g == 2:
                    load_wo(0)
                    load_wo(1)
                elif g == 3:
                    load_wo(2)
                    load_wo(3)
                att_cur = Attn(g)
                # ~31 pump points per group; spread the AV work evenly
                n_av = 1 if att_prev is None else -(-len(att_prev.av_seq) // 20)
                pump = make_pump(att_cur, att_prev, n_av)
                pump_q = make_pump(att_cur, att_prev, n_av,
                                   allow_prev_sc=False)
                if g == 0:
                    rope_q, rope_k = emit_qk_quad(g, xs_c, pump_q)
                    # rope_k must wait for window 1: its ck/sk tables load
                    # after xs-g1 in the DMA priority order
                    pending_rope[0] = rope_k
                    emit_v_pair(g, (0, 1), xs_c, pump, pres=(rope_q,))
                    emit_v_pair(g, (2, 3), xs_c, pump)
                else:
                    if pending_rope[0] is not None:
                        pending_rope[0]()
                        pending_rope[0] = None
                    rope_q = emit_qk_pair(g, "q", xs_c, pump_q)
                    att_prev.stage = 2  # prev group's ropes are done now

                    def _pre_k(rq=rope_q):
                        rq()
                        att_cur.stage = 1
                    rope_k = emit_qk_pair(g, "k", xs_c, pump, pre=_pre_k)

                    def _pre_v(rk=rope_k):
                        rk()
                        att_cur.stage = 2
                    emit_v_pair(g, (0, 1), xs_c, pump, pres=(_pre_v,))
                    emit_v_pair(g, (2, 3), xs_c, pump)
                # drain: finish prev group's scores+AVs; last window also
                # drains its own scores (needed before the tail AVs)
                if att_prev is not None:
                    while att_prev.sc_avail():
                        att_prev.sc_step()
                    while not att_prev.done():
                        att_prev.av_step()
                    att_prev.epi()
                if g == NQG - 1:
                    while att_cur.sc_avail():
                        att_cur.sc_step()
                att_prev = att_cur

            if DBG == "2":
                with tc.tile_pool(name="dbg", bufs=1) as dbgp:
                    for nm, t in (("dbg_qt", qt[0]), ("dbg_kt", kt_[0]),
                                  ("dbg_o", o_sb[0])):
                        tt = dbgp.tile([P, S], f32, tag=nm, name=nm)
                        nc.vector.tensor_copy(tt[:], t[:])
                        nc.scalar.dma_start(locals()[nm] if False else {"dbg_qt": dbg_qt, "dbg_kt": dbg_kt, "dbg_o": dbg_o}[nm][:], tt[:])
                    tt = dbgp.tile([P, NKT, NH], f32, tag="dv", name="dv")
                    nc.vector.tensor_copy(tt[:], vt[:])
                    nc.scalar.dma_start(
                        dbg_vt.rearrange("p (a b) -> p a b", a=NKT)[:], tt[:])
            ps_ctx.__exit__(None, None, None)
            ps_r_ctx = tc.tile_pool(name="ps_r", bufs=B("BPR", 2),
                                    space="PSUM")
            ps_r = ps_r_ctx.__enter__()
            # attention group 3: AV steps, with the last few + epilogue
            # hidden under the first 12 jt of ph4 wave (0, h0)
            while att_prev.av_i < len(att_prev.av_seq) - 2:
                att_prev.av_step()

            def tail_pump():
                while not att_prev.done():
                    att_prev.av_step()
                att_prev.epi()

            ph4_wave(0, 0, tail_pump)
            ph4_wave(0, 1)
            for mg in range(1, NQG):
                for h in range(HPC):
                    ph4_wave(mg, h)
            ps_r_ctx.__exit__(None, None, None)

    nc.compile()
    return nc


def _get_nc(key):
    k = ("nc2", key)
    if k not in _CACHE:
        _CACHE[k] = _build(*key)
    return _CACHE[k]


def _prep_inputs(x, freqs_cos, freqs_sin, mask, wq, wk, wv, wo):
    from concourse import mybir
    f = np.float32
    bf = mybir.dt.np(mybir.dt.bfloat16)
    x = np.asarray(x, f).reshape(S, D)
    mask = np.asarray(mask, f).reshape(S, S)
    wq, wk, wv, wo = (np.asarray(w, f) for w in (wq, wk, wv, wo))
    cos = np.asarray(freqs_cos, f)
    sin = np.asarray(freqs_sin, f)

    xT = np.ascontiguousarray(x.T).astype(bf)
    maskT = np.ascontiguousarray(mask.T)
    woT = np.ascontiguousarray(wo.T).astype(bf)

    C = np.repeat(cos.T, 2, axis=0)          # [128, S], rows 2j,2j+1 = cos_j
    Sg = np.repeat(sin.T, 2, axis=0)
    Sg[0::2] *= -1.0                          # even rows: -sin, odd: +sin
    scale = 1.0 / math.sqrt(DH)
    common = {
        "xT": xT, "maskT": maskT, "woT": woT,
        "cq": np.ascontiguousarray(C * scale).astype(bf),
        "sq": np.ascontiguousarray(Sg * scale).astype(bf),
        "ck": np.ascontiguousarray(C).astype(bf),
        "sk": np.ascontiguousarray(Sg).astype(bf),
        "onesb": np.ones((P, 1), bf),
        "perm": np.eye(P, dtype=f)[
            [i ^ 1 for i in range(P)]].astype(bf),
    }
    in_maps = []
    for c in range(N_CORES):
        rows = slice(c * NH, (c + 1) * NH)
        in_maps.append(dict(
            common,
            wqT=np.ascontiguousarray(wq[rows].T).astype(bf),
            wkT=np.ascontiguousarray(wk[rows].T).astype(bf),
            wvT=np.ascontiguousarray(wv[rows].T).astype(bf),
        ))
    return in_maps


def kernel(x, freqs_cos, freqs_sin, mask, wq, wk, wv, wo, start_pos):
    from concourse.bass_utils import run_bass_kernel_spmd

    in_maps = _prep_inputs(x, freqs_cos, freqs_sin, mask, wq, wk, wv, wo)
    nc = _get_nc(_classify(in_maps[0]["maskT"]))
    res = run_bass_kernel_spmd(nc, in_maps, core_ids=list(range(N_CORES)))
    full = np.concatenate([res.results[c]["out"] for c in range(N_CORES)],
                          axis=0)
    return full.reshape(1, S, D).astype(np.float32)
